# revision 1
# baseline (speedup 1.0000x reference)
"""M2MRF module as a two-GEMM chained Bass kernel on 8 TRN2 NeuronCores.

Math (per batch b of 4):
    cols = unfold(x[b], k=4, s=4)            # [1024, 16384]
    y1   = W1 @ cols + b1                    # [1024, 16384]
    y2   = W2 @ y1 + b2                      # [256, 16384]
    out[b] = fold(y2, k=2, s=2)              # [64, 256, 256]

Sharding: 8 cores = 4 batches x 2 L-halves (L = 16384 patch positions).
Each core runs GEMM1 (1024x1024x8192) + GEMM2 (256x1024x8192) in bf16
with fp32 PSUM accumulation. Unfold/fold are pure data-movement and run
on the host; the device sees contiguous [K, L] operands resident in SBUF.
"""
import sys

sys.path.insert(0, "/opt/trn_rl_repo")

import numpy as np
import ml_dtypes

import concourse.bass as bass
import concourse.bacc as bacc
import concourse.mybir as mybir
import concourse.tile as tile
from concourse.bass_utils import run_bass_kernel_spmd

P = 128
NT = 512            # free-dim tile (one PSUM bank of fp32)
LSH = 8192          # L per core
NTILES = LSH // NT  # 16
KC = 8              # 1024 / 128 contraction chunks
FC = 1024
COUT = 256

_BF16 = ml_dtypes.bfloat16


def _build_nc(ntiles=NTILES):
    nc = bacc.Bacc("TRN2", target_bir_lowering=False)
    xc_dram = [
        nc.dram_tensor(f"xc{k}", [P, LSH], mybir.dt.bfloat16, kind="ExternalInput")
        for k in range(KC)
    ]
    w1_dram = nc.dram_tensor("w1t", [KC, P, FC], mybir.dt.bfloat16, kind="ExternalInput")  # [m, p, k*128+j]
    w2_dram = nc.dram_tensor("w2t", [KC, P, COUT], mybir.dt.bfloat16, kind="ExternalInput")
    y2_dram = nc.dram_tensor("y2", [2, P, LSH], mybir.dt.float32, kind="ExternalOutput")

    with tile.TileContext(nc) as tc:
        with (
            tc.tile_pool(name="resident", bufs=1) as res,
            tc.tile_pool(name="work", bufs=2) as work,
            tc.tile_pool(name="outp", bufs=3) as outp,
            tc.tile_pool(name="ps1", bufs=4, space="PSUM") as ps1,
            tc.tile_pool(name="ps2", bufs=2, space="PSUM") as ps2,
        ):
            w1_sb = res.tile([P, KC, FC], mybir.dt.bfloat16, tag="w1")
            w2_sb = res.tile([P, KC, COUT], mybir.dt.bfloat16, tag="w2")
            xc_sb = [
                res.tile([P, LSH], mybir.dt.bfloat16, tag=f"xc{k}", name=f"xc{k}")
                for k in range(KC)
            ]
            # Issue order tracks first use: the opening m-group of tile 0 needs
            # only W1's m=0 slice plus the head slice of every x chunk.
            nc.sync.dma_start(w1_sb[:, 0, :], w1_dram.ap()[0])
            hsl = slice(0, LSH // 8)
            for k in range(KC):
                nc.sync.dma_start(xc_sb[k][:, hsl], xc_dram[k].ap()[:, hsl])
            for m in range(1, KC):
                nc.sync.dma_start(w1_sb[:, m, :], w1_dram.ap()[m])
            nc.sync.dma_start(w2_sb[:], w2_dram.ap().rearrange("k p m -> p k m"))
            for h in range(1, 8):
                sl = slice(h * (LSH // 8), (h + 1) * (LSH // 8))
                for k in range(KC):
                    nc.sync.dma_start(xc_sb[k][:, sl], xc_dram[k].ap()[:, sl])

            for nt in range(ntiles):
                nsl = slice(nt * NT, (nt + 1) * NT)
                y1_sb = work.tile([P, KC, NT], mybir.dt.bfloat16, tag="y1")
                # GEMM1: y1[m,:] = sum_k W1T[k,:,m]^T @ xc[k][:, nsl]
                for m in range(KC):
                    pt = ps1.tile([P, NT], mybir.dt.float32, tag="ps1")
                    for k in range(KC):
                        nc.tensor.matmul(
                            pt[:],
                            w1_sb[:, m, k * P:(k + 1) * P],
                            xc_sb[k][:, nsl],
                            start=(k == 0),
                            stop=(k == KC - 1),
                        )
                    nc.vector.tensor_copy(y1_sb[:, m, :], pt[:])
                # GEMM2: y2[m2,:] = sum_k W2T[k,:,m2]^T @ y1[k,:]
                o_sb = outp.tile([P, 2, NT], mybir.dt.float32, tag="o")
                for m2 in range(2):
                    pt2 = ps2.tile([P, NT], mybir.dt.float32, tag="ps2")
                    for k in range(KC):
                        nc.tensor.matmul(
                            pt2[:],
                            w2_sb[:, k, m2 * P:(m2 + 1) * P],
                            y1_sb[:, k, :],
                            start=(k == 0),
                            stop=(k == KC - 1),
                        )
                    nc.any.tensor_copy(out=o_sb[:, m2, :], in_=pt2[:])
                    nc.sync.dma_start(y2_dram.ap()[m2, :, nsl], o_sb[:, m2, :])

    nc.finalize()
    return nc


_NC_CACHE = None


def kernel(x, W1, b1, W2, b2):
    global _NC_CACHE
    x = np.asarray(x)
    W1, b1 = np.asarray(W1), np.asarray(b1)
    W2, b2 = np.asarray(W2), np.asarray(b2)
    n, c, h, w = x.shape  # 4, 64, 512, 512

    # ---- host unfold: cols[b, c*16+kh*4+kw, ph*128+pw] = x[b,c,ph*4+kh,pw*4+kw]
    xb = x.astype(_BF16)
    cols = xb.reshape(n, c, 128, 4, 128, 4).transpose(0, 1, 3, 5, 2, 4)
    cols = np.ascontiguousarray(cols).reshape(n, 1024, 16384)

    w1t = np.ascontiguousarray(
        W1.astype(_BF16).reshape(KC, P, KC, P).transpose(0, 3, 2, 1)
    ).reshape(KC, P, FC)
    w2t = np.ascontiguousarray(W2.T.astype(_BF16)).reshape(KC, P, COUT)

    if _NC_CACHE is None:
        _NC_CACHE = _build_nc()
    nc = _NC_CACHE

    in_maps = []
    for core in range(8):
        b, half = core // 2, core % 2
        xc = np.ascontiguousarray(
            cols[b, :, half * LSH:(half + 1) * LSH]
        ).reshape(KC, P, LSH)
        m = {f"xc{k}": xc[k] for k in range(KC)}
        m["w1t"] = w1t
        m["w2t"] = w2t
        in_maps.append(m)

    res = run_bass_kernel_spmd(nc, in_maps, core_ids=list(range(8)))

    # ---- gather + fold on host
    y2 = np.empty((n, COUT, 16384), dtype=np.float32)
    for core in range(8):
        b, half = core // 2, core % 2
        y2[b, :, half * LSH:(half + 1) * LSH] = (
            res.results[core]["y2"].reshape(COUT, LSH)
        )

    # bias epilogue (b1/b2 are zeros in this problem; exact otherwise)
    v = W2.astype(np.float64) @ b1.astype(np.float64) + b2.astype(np.float64)
    if np.any(v):
        y2 += v.astype(np.float32)[None, :, None]

    out = y2.reshape(n, c, 2, 2, 128, 128).transpose(0, 1, 4, 2, 5, 3)
    return np.ascontiguousarray(out).reshape(n, c, 256, 256)



# revision 2
# speedup vs baseline: 2.0211x; 2.0211x over previous
"""M2MRF module as a single collapsed GEMM on 8 TRN2 NeuronCores.

The reference is fold(W2 @ (W1 @ unfold(x) + b1) + b2) -- two chained
linear maps with NO nonlinearity between them, so the device only needs
the collapsed weight Wc = W2 @ W1 (precomputed on host in float64):

    cols  = unfold(x[b], k=4, s=4)        # [1024, 16384]
    y2    = Wc @ cols                     # [256, 16384]  (bias via host epilogue)
    out[b] = fold(y2, k=2, s=2)           # [64, 256, 256]

This is a 5x FLOP reduction vs the chained two-GEMM formulation
(1024x1024 + 256x1024 -> 256x1024 per column).

Sharding: 8 cores = 4 batches x 2 L-halves (L = 16384 patch positions).
Each core runs one GEMM (256 x 1024 x 8192) in bf16 with fp32 PSUM
accumulation; output returns in bf16 (error budget allows it), halving
output DMA. Unfold/fold are pure data movement and run on the host.

Per-core roofline: 2*8*8192 = 131072 PE cycles @ 2.4 GHz = 54.6 us
compute; 16.8 MB in + 0.5 MB weights + 4.2 MB out = 21.5 MB DMA
@ 360 GB/s = 59.7 us -> DMA-bus-bound at ~61 us.
"""
import sys

sys.path.insert(0, "/opt/trn_rl_repo")

import numpy as np
import ml_dtypes

import concourse.bass as bass
import concourse.bacc as bacc
import concourse.mybir as mybir
import concourse.tile as tile
from concourse.bass_utils import run_bass_kernel_spmd

P = 128
NT = 512            # free-dim tile (one PSUM bank of fp32)
LSH = 8192          # L per core
NTILES = LSH // NT  # 16
KC = 8              # 1024 / 128 contraction chunks
MC = 2              # 256 / 128 output chunks
COUT = 256

_BF16 = ml_dtypes.bfloat16


def _build_nc(ntiles=NTILES):
    nc = bacc.Bacc("TRN2", target_bir_lowering=False)
    xc_dram = [
        nc.dram_tensor(f"xc{k}", [P, LSH], mybir.dt.bfloat16, kind="ExternalInput")
        for k in range(KC)
    ]
    # Partition-major weight layout: wct[p, m, k, j] = Wc[m*128+j, k*128+p]
    # so each m-half DMAs as one contiguous 2KB-per-partition transfer.
    wct_dram = nc.dram_tensor("wct", [P, MC, KC, P], mybir.dt.bfloat16, kind="ExternalInput")
    y2_dram = nc.dram_tensor("y2", [MC, P, LSH], mybir.dt.bfloat16, kind="ExternalOutput")

    with tile.TileContext(nc) as tc:
        with (
            tc.tile_pool(name="resident", bufs=1) as res,
            tc.tile_pool(name="outp", bufs=3) as outp,
            tc.tile_pool(name="ps", bufs=4, space="PSUM") as ps,
        ):
            wc_sb = res.tile([P, MC, KC, P], mybir.dt.bfloat16, tag="wc")
            xc_sb = [
                res.tile([P, LSH], mybir.dt.bfloat16, tag=f"xc{k}", name=f"xc{k}")
                for k in range(KC)
            ]
            # Issue order tracks first use: m=0 weights, then the head
            # 512-col slice of every x chunk (enough for tile 0), then m=1
            # weights, then the remaining x slices in column order.
            nc.sync.dma_start(wc_sb[:, 0], wct_dram.ap()[:, 0])
            fsl = slice(0, NT)
            for k in range(KC):
                nc.sync.dma_start(xc_sb[k][:, fsl], xc_dram[k].ap()[:, fsl])
            nc.sync.dma_start(wc_sb[:, 1], wct_dram.ap()[:, 1])
            for s in range(1, ntiles):
                sl = slice(s * NT, (s + 1) * NT)
                for k in range(KC):
                    nc.sync.dma_start(xc_sb[k][:, sl], xc_dram[k].ap()[:, sl])

            for nt in range(ntiles):
                nsl = slice(nt * NT, (nt + 1) * NT)
                o_sb = outp.tile([P, MC, NT], mybir.dt.bfloat16, tag="o")
                for m in range(MC):
                    pt = ps.tile([P, NT], mybir.dt.float32, tag="ps")
                    for k in range(KC):
                        nc.tensor.matmul(
                            pt[:],
                            wc_sb[:, m, k, :],
                            xc_sb[k][:, nsl],
                            start=(k == 0),
                            stop=(k == KC - 1),
                        )
                    nc.any.tensor_copy(out=o_sb[:, m], in_=pt[:])
                    nc.sync.dma_start(y2_dram.ap()[m, :, nsl], o_sb[:, m])

    nc.finalize()
    return nc


_NC_CACHE = None


def kernel(x, W1, b1, W2, b2):
    global _NC_CACHE
    x = np.asarray(x)
    W1, b1 = np.asarray(W1), np.asarray(b1)
    W2, b2 = np.asarray(W2), np.asarray(b2)
    n, c, h, w = x.shape  # 4, 64, 512, 512

    # ---- host unfold: cols[b, c*16+kh*4+kw, ph*128+pw] = x[b,c,ph*4+kh,pw*4+kw]
    xb = x.astype(_BF16)
    cols = xb.reshape(n, c, 128, 4, 128, 4).transpose(0, 1, 3, 5, 2, 4)
    cols = np.ascontiguousarray(cols).reshape(n, 1024, 16384)

    # ---- collapsed weight (exact in f64, one bf16 rounding)
    Wc = (W2.astype(np.float64) @ W1.astype(np.float64))  # [256, 1024]
    wct = np.ascontiguousarray(
        Wc.reshape(MC, P, KC, P).transpose(3, 0, 2, 1)
    ).astype(_BF16)  # [p, m, k, j]

    if _NC_CACHE is None:
        _NC_CACHE = _build_nc()
    nc = _NC_CACHE

    in_maps = []
    for core in range(8):
        b, half = core // 2, core % 2
        xc = np.ascontiguousarray(
            cols[b, :, half * LSH:(half + 1) * LSH]
        ).reshape(KC, P, LSH)
        m = {f"xc{k}": xc[k] for k in range(KC)}
        m["wct"] = wct
        in_maps.append(m)

    res = run_bass_kernel_spmd(nc, in_maps, core_ids=list(range(8)))

    # ---- gather + fold on host
    y2 = np.empty((n, COUT, 16384), dtype=np.float32)
    for core in range(8):
        b, half = core // 2, core % 2
        y2[b, :, half * LSH:(half + 1) * LSH] = (
            res.results[core]["y2"].reshape(COUT, LSH).astype(np.float32)
        )

    # bias epilogue (b1/b2 are zeros in this problem; exact otherwise)
    v = W2.astype(np.float64) @ b1.astype(np.float64) + b2.astype(np.float64)
    if np.any(v):
        y2 += v.astype(np.float32)[None, :, None]

    out = y2.reshape(n, c, 2, 2, 128, 128).transpose(0, 1, 4, 2, 5, 3)
    return np.ascontiguousarray(out).reshape(n, c, 256, 256)


# revision 3
# speedup vs baseline: 4.0426x; 2.0002x over previous
"""M2MRF module as a single collapsed GEMM on 8 TRN2 NeuronCores.

The reference is fold(W2 @ (W1 @ unfold(x) + b1) + b2) -- two chained
linear maps with NO nonlinearity between them, so the device only needs
the collapsed weight Wc = W2 @ W1 (precomputed on host in float64):

    cols  = unfold(x[b], k=4, s=4)        # [1024, 16384]
    y2    = Wc @ cols                     # [256, 16384]  (bias via host epilogue)
    out[b] = fold(y2, k=2, s=2)           # [64, 256, 256]

This is a 5x FLOP reduction vs the chained two-GEMM formulation.

Sharding: 8 cores = 4 batches x 2 L-halves (L = 16384 patch positions).
Each core runs one GEMM (256 x 1024 x 8192) in bf16 with fp32 PSUM
accumulation; output returns in bf16 (error budget allows it), halving
output DMA.

DMA-instruction count is the second-order constraint: each DMA holds the
global HWDGE descriptor generator ~650ns, so inputs are host-packed into
16 column-slice DMAs (each carrying all 8 K-chunks), 1 weight DMA, and
16 output DMAs (one per L-tile).

Per-core roofline: 2*8*8192 = 131072 PE cycles @ 2.4 GHz = 54.6 us
compute; 16.8 MB in + 0.5 MB weights + 4.2 MB out = 21.5 MB DMA
@ 360 GB/s = 59.7 us bus -> ~62 us overall.
"""
import sys

sys.path.insert(0, "/opt/trn_rl_repo")

import numpy as np
import ml_dtypes

import concourse.bass as bass
import concourse.bacc as bacc
import concourse.mybir as mybir
import concourse.tile as tile
from concourse.bass_utils import run_bass_kernel_spmd

P = 128
NT = 512            # free-dim tile (one PSUM bank of fp32)
LSH = 8192          # L per core
NTILES = LSH // NT  # 16
KC = 8              # 1024 / 128 contraction chunks
MC = 2              # 256 / 128 output chunks
COUT = 256

_BF16 = ml_dtypes.bfloat16


def _build_nc(ntiles=NTILES):
    nc = bacc.Bacc("TRN2", target_bir_lowering=False)
    # xs[s][p, k*NT+j] = cols[k*128+p, s*NT+j]: one DMA per column slice s
    # delivers the slice of every contraction chunk k.
    xs_dram = nc.dram_tensor("xs", [ntiles, P, KC * NT], mybir.dt.bfloat16,
                             kind="ExternalInput")
    # wct[p, m, k, j] = Wc[m*128+j, k*128+p]: contiguous 4KB per partition.
    wct_dram = nc.dram_tensor("wct", [P, MC, KC, P], mybir.dt.bfloat16,
                              kind="ExternalInput")
    # y2[s, p, m, j] = y2_full[m*128+p, s*NT+j]: contiguous 2KB per partition.
    y2_dram = nc.dram_tensor("y2", [ntiles, P, MC, NT], mybir.dt.bfloat16,
                             kind="ExternalOutput")

    with tile.TileContext(nc) as tc:
        with (
            tc.tile_pool(name="resident", bufs=1) as res,
            tc.tile_pool(name="ps", bufs=4, space="PSUM") as ps,
        ):
            wc_sb = res.tile([P, MC, KC, P], mybir.dt.bfloat16, tag="wc")
            xc_sb = res.tile([P, KC, LSH], mybir.dt.bfloat16, tag="xc")
            # All output tiles resident so compute never stalls on output
            # DMA drain (output transfers queue behind input transfers on
            # the DMA bus).
            o_sb = res.tile([P, ntiles, MC, NT], mybir.dt.bfloat16, tag="o")

            nc.sync.dma_start(wc_sb[:], wct_dram.ap())
            for s in range(ntiles):
                nsl = slice(s * NT, (s + 1) * NT)
                nc.sync.dma_start(
                    xc_sb[:, :, nsl],
                    xs_dram.ap()[s].rearrange("p (k j) -> p k j", k=KC),
                )

            for nt in range(ntiles):
                nsl = slice(nt * NT, (nt + 1) * NT)
                for m in range(MC):
                    pt = ps.tile([P, NT], mybir.dt.float32, tag="ps")
                    for k in range(KC):
                        nc.tensor.matmul(
                            pt[:],
                            wc_sb[:, m, k, :],
                            xc_sb[:, k, nsl],
                            start=(k == 0),
                            stop=(k == KC - 1),
                        )
                    nc.any.tensor_copy(out=o_sb[:, nt, m], in_=pt[:])
                nc.sync.dma_start(y2_dram.ap()[nt], o_sb[:, nt])

    nc.finalize()
    return nc


_NC_CACHE = None


def kernel(x, W1, b1, W2, b2):
    global _NC_CACHE
    x = np.asarray(x)
    W1, b1 = np.asarray(W1), np.asarray(b1)
    W2, b2 = np.asarray(W2), np.asarray(b2)
    n, c, h, w = x.shape  # 4, 64, 512, 512

    # ---- host unfold: cols[b, c*16+kh*4+kw, ph*128+pw] = x[b,c,ph*4+kh,pw*4+kw]
    xb = x.astype(_BF16)
    cols = xb.reshape(n, c, 128, 4, 128, 4).transpose(0, 1, 3, 5, 2, 4)
    cols = np.ascontiguousarray(cols).reshape(n, 1024, 16384)

    # ---- collapsed weight (exact in f64, one bf16 rounding)
    Wc = W2.astype(np.float64) @ W1.astype(np.float64)  # [256, 1024]
    wct = np.ascontiguousarray(
        Wc.reshape(MC, P, KC, P).transpose(3, 0, 2, 1)
    ).astype(_BF16)  # [p, m, k, j]

    if _NC_CACHE is None:
        _NC_CACHE = _build_nc()
    nc = _NC_CACHE

    in_maps = []
    for core in range(8):
        b, half = core // 2, core % 2
        xc = cols[b, :, half * LSH:(half + 1) * LSH]  # [1024, LSH]
        # [NTILES, P, KC, NT]: xs[s, p, k, j] = xc[k*128+p, s*NT+j]
        xs = np.ascontiguousarray(
            xc.reshape(KC, P, NTILES, NT).transpose(2, 1, 0, 3)
        ).reshape(NTILES, P, KC * NT)
        in_maps.append({"xs": xs, "wct": wct})

    res = run_bass_kernel_spmd(nc, in_maps, core_ids=list(range(8)))

    # ---- gather + fold on host
    y2 = np.empty((n, COUT, 16384), dtype=np.float32)
    for core in range(8):
        b, half = core // 2, core % 2
        arr = res.results[core]["y2"]  # [NTILES, P, MC, NT]
        y2[b, :, half * LSH:(half + 1) * LSH] = (
            arr.transpose(2, 1, 0, 3).reshape(COUT, LSH).astype(np.float32)
        )

    # bias epilogue (b1/b2 are zeros in this problem; exact otherwise)
    v = W2.astype(np.float64) @ b1.astype(np.float64) + b2.astype(np.float64)
    if np.any(v):
        y2 += v.astype(np.float32)[None, :, None]

    out = y2.reshape(n, c, 2, 2, 128, 128).transpose(0, 1, 4, 2, 5, 3)
    return np.ascontiguousarray(out).reshape(n, c, 256, 256)


# revision 14
# speedup vs baseline: 4.4070x; 1.0901x over previous
"""M2MRF module as a single collapsed GEMM on 8 TRN2 NeuronCores.

The reference is fold(W2 @ (W1 @ unfold(x) + b1) + b2) -- two chained
linear maps with NO nonlinearity between them, so the device only needs
the collapsed weight Wc = W2 @ W1 (precomputed on host in float64):

    cols  = unfold(x[b], k=4, s=4)        # [1024, 16384]
    y2    = Wc @ cols                     # [256, 16384]  (bias via host epilogue)
    out[b] = fold(y2, k=2, s=2)           # [64, 256, 256]

This is a 5x FLOP reduction vs the chained two-GEMM formulation.

Sharding: 8 cores = 4 batches x 2 L-halves (L = 16384 patch positions).
Each core runs one GEMM (256 x 1024 x 8192) in bf16 with fp32 PSUM
accumulation; output returns in bf16 (error budget allows it).

Schedule notes (from timeline analysis):
  - Each DMA holds the global HWDGE descriptor generator ~650ns, so
    inputs are packed into 256-column slices each carrying all 8
    K-chunks (one DMA per slice), 1 weight DMA, ~17 output DMAs.
  - The PE p-state ramp (1.2 GHz for the first 3us after any idle
    period) is bridged with warmup matmuls on a zeroed tile so real
    matmuls all run at 2.4 GHz.
  - 256-column slices let compute start ~5us in and keep slice arrival
    (1.46us) ahead of consumption (1.71us).
  - The last 512-tile is split in two so the final copy+DMA tail is
    short.

Per-core roofline: 2*8*8192 = 131072 PE cycles @ 2.4 GHz = 54.6 us
compute; 21.5 MB total DMA @ 360 GB/s = 59.7 us bus (finishes early,
not critical). Target ~= 5 + 54.6 + 3.8 = 63 us.
"""
import sys

sys.path.insert(0, "/opt/trn_rl_repo")

import numpy as np
import ml_dtypes

import concourse.bass as bass
import concourse.bacc as bacc
import concourse.mybir as mybir
import concourse.tile as tile
from concourse.bass_utils import run_bass_kernel_spmd

P = 128
NT = 512            # PSUM tile free dim
NS = 256            # input slice cols
LSH = 8192          # L per core
NTILES = LSH // NT  # 16
NSLICES = LSH // NS # 32
KC = 8              # 1024 / 128 contraction chunks
MC = 2              # 256 / 128 output chunks
COUT = 256

WARMUP_FULL = 9     # warmup matmuls of 512 cols
WARMUP_FINE = 0     # warmup matmuls of 128 cols (fine-grained bridge tail)
NTAIL = 128         # final block, DMA'd straight from PSUM as fp32

_BF16 = ml_dtypes.bfloat16


def _build_nc(warmup_full=WARMUP_FULL, warmup_fine=WARMUP_FINE):
    nc = bacc.Bacc("TRN2", target_bir_lowering=False)
    # xs[s][p, k*NS+j] = cols[k*128+p, s*NS+j]: one DMA per column slice s
    # delivers the slice of every contraction chunk k.
    xs_dram = nc.dram_tensor("xs", [NSLICES, P, KC * NS], mybir.dt.bfloat16,
                             kind="ExternalInput")
    # wct[p, m, k, j] = Wc[m*128+j, k*128+p]: contiguous 4KB per partition.
    wct_dram = nc.dram_tensor("wct", [P, MC, KC, P], mybir.dt.bfloat16,
                              kind="ExternalInput")
    # y2[t, p, m, j] = y2_full[m*128+p, t*NT+j]: contiguous 2KB per partition.
    y2_dram = nc.dram_tensor("y2", [NTILES, P, MC, NT], mybir.dt.bfloat16,
                             kind="ExternalOutput")

    with tile.TileContext(nc) as tc:
        with (
            tc.tile_pool(name="resident", bufs=1) as res,
            tc.tile_pool(name="ps", bufs=3, space="PSUM") as ps,
            tc.tile_pool(name="psw", bufs=1, space="PSUM") as psw,
        ):
            wz = res.tile([P, NT], mybir.dt.bfloat16, tag="wz")
            wc_sb = res.tile([P, MC, KC, P], mybir.dt.bfloat16, tag="wc")
            xc_sb = res.tile([P, KC, LSH], mybir.dt.bfloat16, tag="xc")
            # All output tiles resident so compute never stalls on output
            # DMA drain (output transfers queue behind input transfers on
            # the DMA bus).
            o_sb = res.tile([P, NTILES, MC, NT], mybir.dt.bfloat16, tag="o")

            # PE warmup: matmuls on a zeroed tile bridge the p-state ramp
            # so the PE is at 2.4 GHz when the first real operands land.
            nc.vector.memset(wz[:], 0.0)
            pw = psw.tile([P, NT], mybir.dt.float32, tag="pw")
            for i in range(warmup_full):
                nc.tensor.matmul(pw[:], wz[:, 0:P], wz[:], start=True, stop=True)
            for i in range(warmup_fine):
                nc.tensor.matmul(pw[:, 0:P], wz[:, 0:P], wz[:, 0:P],
                                 start=True, stop=True)

            nc.sync.dma_start(wc_sb[:], wct_dram.ap())
            for s in range(NSLICES):
                csl = slice(s * NS, (s + 1) * NS)
                nc.sync.dma_start(
                    xc_sb[:, :, csl],
                    xs_dram.ap()[s].rearrange("p (k j) -> p k j", k=KC),
                )

            # Compute in 256-col blocks; psum tiles span 512 cols (2 blocks).
            for t in range(NTILES - 1):
                pt = [ps.tile([P, NT], mybir.dt.float32, tag=f"ps{m}",
                              name=f"pt{t}_{m}")
                      for m in range(MC)]
                for h in range(2):  # half = 256-col block = one input slice
                    hsl = slice(t * NT + h * NS, t * NT + (h + 1) * NS)
                    psl = slice(h * NS, (h + 1) * NS)
                    for m in range(MC):
                        for k in range(KC):
                            nc.tensor.matmul(
                                pt[m][:, psl],
                                wc_sb[:, m, k, :],
                                xc_sb[:, k, hsl],
                                start=(k == 0),
                                stop=(k == KC - 1),
                            )
                for m in range(MC):
                    nc.any.tensor_copy(out=o_sb[:, t, m], in_=pt[m][:])
                nc.sync.dma_start(y2_dram.ap()[t], o_sb[:, t])

            # Last tile: a 384-col block through the normal copy path, then
            # a final NTAIL-col block accumulated in its own PSUM tile and
            # DMA'd straight to DRAM (fp32) so the tail is one short chain.
            t = NTILES - 1
            NH = NT - NTAIL  # 384
            pt = [ps.tile([P, NT], mybir.dt.float32, tag=f"ps{m}",
                          name=f"ptl{m}")
                  for m in range(MC)]
            for m in range(MC):
                for k in range(KC):
                    nc.tensor.matmul(
                        pt[m][:, 0:NH],
                        wc_sb[:, m, k, :],
                        xc_sb[:, k, t * NT:t * NT + NH],
                        start=(k == 0),
                        stop=(k == KC - 1),
                    )
            nc.scalar.copy(out=o_sb[:, t, 0, 0:NH], in_=pt[0][:, 0:NH])
            nc.vector.tensor_copy(out=o_sb[:, t, 1, 0:NH], in_=pt[1][:, 0:NH])
            nc.sync.dma_start(y2_dram.ap()[t, :, :, 0:NH], o_sb[:, t, :, 0:NH])

            ptail = psw.tile([P, MC, NTAIL], mybir.dt.float32, tag="ptail")
            for m in range(MC):
                for k in range(KC):
                    nc.tensor.matmul(
                        ptail[:, m, :],
                        wc_sb[:, m, k, :],
                        xc_sb[:, k, LSH - NTAIL:LSH],
                        start=(k == 0),
                        stop=(k == KC - 1),
                    )
            nc.scalar.copy(out=o_sb[:, t, :, NH:NT], in_=ptail[:])
            nc.sync.dma_start(y2_dram.ap()[t, :, :, NH:NT], o_sb[:, t, :, NH:NT])

    nc.finalize()
    return nc


_NC_CACHE = None


def kernel(x, W1, b1, W2, b2):
    global _NC_CACHE
    x = np.asarray(x)
    W1, b1 = np.asarray(W1), np.asarray(b1)
    W2, b2 = np.asarray(W2), np.asarray(b2)
    n, c, h, w = x.shape  # 4, 64, 512, 512

    # ---- host unfold: cols[b, c*16+kh*4+kw, ph*128+pw] = x[b,c,ph*4+kh,pw*4+kw]
    xb = x.astype(_BF16)
    cols = xb.reshape(n, c, 128, 4, 128, 4).transpose(0, 1, 3, 5, 2, 4)
    cols = np.ascontiguousarray(cols).reshape(n, 1024, 16384)

    # ---- collapsed weight (exact in f64, one bf16 rounding)
    Wc = W2.astype(np.float64) @ W1.astype(np.float64)  # [256, 1024]
    wct = np.ascontiguousarray(
        Wc.reshape(MC, P, KC, P).transpose(3, 0, 2, 1)
    ).astype(_BF16)  # [p, m, k, j]

    if _NC_CACHE is None:
        _NC_CACHE = _build_nc()
    nc = _NC_CACHE

    in_maps = []
    for core in range(8):
        b, half = core // 2, core % 2
        xc = cols[b, :, half * LSH:(half + 1) * LSH]  # [1024, LSH]
        # [NSLICES, P, KC, NS]: xs[s, p, k, j] = xc[k*128+p, s*NS+j]
        xs = np.ascontiguousarray(
            xc.reshape(KC, P, NSLICES, NS).transpose(2, 1, 0, 3)
        ).reshape(NSLICES, P, KC * NS)
        in_maps.append({"xs": xs, "wct": wct})

    res = run_bass_kernel_spmd(nc, in_maps, core_ids=list(range(8)))

    # ---- gather + fold on host
    y2 = np.empty((n, COUT, 16384), dtype=np.float32)
    for core in range(8):
        b, half = core // 2, core % 2
        arr = res.results[core]["y2"]  # [NTILES, P, MC, NT]
        y2[b, :, half * LSH:(half + 1) * LSH] = (
            arr.transpose(2, 1, 0, 3).reshape(COUT, LSH).astype(np.float32)
        )

    # bias epilogue (b1/b2 are zeros in this problem; exact otherwise)
    v = W2.astype(np.float64) @ b1.astype(np.float64) + b2.astype(np.float64)
    if np.any(v):
        y2 += v.astype(np.float32)[None, :, None]

    out = y2.reshape(n, c, 2, 2, 128, 128).transpose(0, 1, 4, 2, 5, 3)
    return np.ascontiguousarray(out).reshape(n, c, 256, 256)


# revision 22
# speedup vs baseline: 4.4384x; 1.0071x over previous
"""M2MRF module as a single collapsed GEMM on 8 TRN2 NeuronCores.

The reference is fold(W2 @ (W1 @ unfold(x) + b1) + b2) -- two chained
linear maps with NO nonlinearity between them, so the device only needs
the collapsed weight Wc = W2 @ W1 (precomputed on host in float64):

    cols  = unfold(x[b], k=4, s=4)        # [1024, 16384]
    y2    = Wc @ cols                     # [256, 16384]  (bias via host epilogue)
    out[b] = fold(y2, k=2, s=2)           # [64, 256, 256]

This is a 5x FLOP reduction vs the chained two-GEMM formulation.

Sharding: 8 cores = 4 batches x 2 L-halves (L = 16384 patch positions).
Each core runs one GEMM (256 x 1024 x 8192) in bf16 with fp32 PSUM
accumulation; output returns in bf16 (error budget allows it).

Schedule notes (from timeline analysis):
  - Each DMA holds the global HWDGE descriptor generator ~650ns, so
    inputs are packed into 256-column slices each carrying all 8
    K-chunks (one DMA per slice), 1 weight DMA, ~17 output DMAs.
  - The PE p-state ramp (1.2 GHz for the first 3us after any idle
    period) is bridged with warmup matmuls on a zeroed tile so real
    matmuls all run at 2.4 GHz.
  - 256-column slices let compute start ~5us in and keep slice arrival
    (1.46us) ahead of consumption (1.71us).
  - The last 512-tile is split in two so the final copy+DMA tail is
    short.

Per-core roofline: 2*8*8192 = 131072 PE cycles @ 2.4 GHz = 54.6 us
compute; 21.5 MB total DMA @ 360 GB/s = 59.7 us bus (finishes early,
not critical). Target ~= 5 + 54.6 + 3.8 = 63 us.
"""
import sys

sys.path.insert(0, "/opt/trn_rl_repo")

import numpy as np
import ml_dtypes

import concourse.bass as bass
import concourse.bacc as bacc
import concourse.mybir as mybir
import concourse.tile as tile
from concourse.bass_utils import run_bass_kernel_spmd

P = 128
NT = 512            # PSUM tile free dim
NS = 256            # input slice cols
LSH = 8192          # L per core
NTILES = LSH // NT  # 16
NSLICES = LSH // NS # 32
KC = 8              # 1024 / 128 contraction chunks
MC = 2              # 256 / 128 output chunks
COUT = 256

WARMUP_FULL = 38    # warmup matmuls of 128 cols
NTAIL = 128         # final block: own PSUM tile, short copy+DMA chain

_BF16 = ml_dtypes.bfloat16


def _build_nc(warmup_full=WARMUP_FULL):
    nc = bacc.Bacc("TRN2", target_bir_lowering=False)
    # xs[s][p, k*NS+j] = cols[k*128+p, s*NS+j]: one DMA per column slice s
    # delivers the slice of every contraction chunk k.
    xs_dram = nc.dram_tensor("xs", [NSLICES, P, KC * NS], mybir.dt.bfloat16,
                             kind="ExternalInput")
    # wct[p, m, k, j] = Wc[m*128+j, k*128+p]: contiguous 4KB per partition.
    wct_dram = nc.dram_tensor("wct", [P, MC, KC, P], mybir.dt.bfloat16,
                              kind="ExternalInput")
    # y2[t, p, m, j] = y2_full[m*128+p, t*NT+j]: contiguous 2KB per partition.
    y2_dram = nc.dram_tensor("y2", [NTILES, P, MC, NT], mybir.dt.bfloat16,
                             kind="ExternalOutput")
    # final NTAIL cols in their own contiguous tensor (512B-elem descriptors)
    y2t_dram = nc.dram_tensor("y2t", [P, MC, NTAIL], mybir.dt.bfloat16,
                              kind="ExternalOutput")

    with tile.TileContext(nc) as tc:
        with (
            tc.tile_pool(name="resident", bufs=1) as res,
            tc.tile_pool(name="ps", bufs=3, space="PSUM") as ps,
            tc.tile_pool(name="psw", bufs=1, space="PSUM") as psw,
        ):
            wz = res.tile([P, NT], mybir.dt.bfloat16, tag="wz")
            wc_sb = res.tile([P, MC, KC, P], mybir.dt.bfloat16, tag="wc")
            xc_sb = res.tile([P, KC, LSH], mybir.dt.bfloat16, tag="xc")
            # All output tiles resident so compute never stalls on output
            # DMA drain (output transfers queue behind input transfers on
            # the DMA bus).
            o_sb = res.tile([P, NTILES, MC, NT], mybir.dt.bfloat16, tag="o")

            # PE warmup: matmuls on a zeroed tile bridge the p-state ramp
            # so the PE is at 2.4 GHz when the first real operands land.
            # Small memset -> warmup starts ~1.1us; 128-col warmups keep the
            # PE busy until the first input slice lands ~5.8us.
            nc.vector.memset(wz[:, 0:P], 0.0)
            pw = psw.tile([P, NT], mybir.dt.float32, tag="pw")
            for i in range(warmup_full):
                nc.tensor.matmul(pw[:, 0:P], wz[:, 0:P], wz[:, 0:P],
                                 start=True, stop=True)

            nc.sync.dma_start(wc_sb[:], wct_dram.ap())
            for s in range(NSLICES):
                csl = slice(s * NS, (s + 1) * NS)
                nc.sync.dma_start(
                    xc_sb[:, :, csl],
                    xs_dram.ap()[s].rearrange("p (k j) -> p k j", k=KC),
                )

            # Compute in 256-col blocks; psum tiles span 512 cols (2 blocks).
            for t in range(NTILES - 1):
                pt = [ps.tile([P, NT], mybir.dt.float32, tag=f"ps{m}",
                              name=f"pt{t}_{m}")
                      for m in range(MC)]
                for h in range(2):  # half = 256-col block = one input slice
                    hsl = slice(t * NT + h * NS, t * NT + (h + 1) * NS)
                    psl = slice(h * NS, (h + 1) * NS)
                    for m in range(MC):
                        for k in range(KC):
                            nc.tensor.matmul(
                                pt[m][:, psl],
                                wc_sb[:, m, k, :],
                                xc_sb[:, k, hsl],
                                start=(k == 0),
                                stop=(k == KC - 1),
                            )
                for m in range(MC):
                    nc.any.tensor_copy(out=o_sb[:, t, m], in_=pt[m][:])
                nc.sync.dma_start(y2_dram.ap()[t], o_sb[:, t])

            # Last tile: a 384-col block through the normal copy path, then
            # a final NTAIL-col block accumulated in its own PSUM tile and
            # DMA'd straight to DRAM (fp32) so the tail is one short chain.
            t = NTILES - 1
            NH = NT - NTAIL  # 384
            pt = [ps.tile([P, NT], mybir.dt.float32, tag=f"ps{m}",
                          name=f"ptl{m}")
                  for m in range(MC)]
            for m in range(MC):
                for k in range(KC):
                    nc.tensor.matmul(
                        pt[m][:, 0:NH],
                        wc_sb[:, m, k, :],
                        xc_sb[:, k, t * NT:t * NT + NH],
                        start=(k == 0),
                        stop=(k == KC - 1),
                    )
            nc.scalar.copy(out=o_sb[:, t, 0, 0:NH], in_=pt[0][:, 0:NH])
            nc.vector.tensor_copy(out=o_sb[:, t, 1, 0:NH], in_=pt[1][:, 0:NH])
            nc.sync.dma_start(y2_dram.ap()[t, :, :, 0:NH], o_sb[:, t, :, 0:NH])

            ptail = psw.tile([P, MC, NTAIL], mybir.dt.float32, tag="ptail")
            ot_sb = res.tile([P, MC, NTAIL], mybir.dt.bfloat16, tag="ot")
            for m in range(MC):
                for k in range(KC):
                    nc.tensor.matmul(
                        ptail[:, m, :],
                        wc_sb[:, m, k, :],
                        xc_sb[:, k, LSH - NTAIL:LSH],
                        start=(k == 0),
                        stop=(k == KC - 1),
                    )
            nc.vector.tensor_copy(out=ot_sb[:], in_=ptail[:])
            nc.sync.dma_start(y2t_dram.ap(), ot_sb[:])

    nc.finalize()
    return nc


_NC_CACHE = None


def kernel(x, W1, b1, W2, b2):
    global _NC_CACHE
    x = np.asarray(x)
    W1, b1 = np.asarray(W1), np.asarray(b1)
    W2, b2 = np.asarray(W2), np.asarray(b2)
    n, c, h, w = x.shape  # 4, 64, 512, 512

    # ---- host unfold: cols[b, c*16+kh*4+kw, ph*128+pw] = x[b,c,ph*4+kh,pw*4+kw]
    xb = x.astype(_BF16)
    cols = xb.reshape(n, c, 128, 4, 128, 4).transpose(0, 1, 3, 5, 2, 4)
    cols = np.ascontiguousarray(cols).reshape(n, 1024, 16384)

    # ---- collapsed weight (exact in f64, one bf16 rounding)
    Wc = W2.astype(np.float64) @ W1.astype(np.float64)  # [256, 1024]
    wct = np.ascontiguousarray(
        Wc.reshape(MC, P, KC, P).transpose(3, 0, 2, 1)
    ).astype(_BF16)  # [p, m, k, j]

    if _NC_CACHE is None:
        _NC_CACHE = _build_nc()
    nc = _NC_CACHE

    in_maps = []
    for core in range(8):
        b, half = core // 2, core % 2
        xc = cols[b, :, half * LSH:(half + 1) * LSH]  # [1024, LSH]
        # [NSLICES, P, KC, NS]: xs[s, p, k, j] = xc[k*128+p, s*NS+j]
        xs = np.ascontiguousarray(
            xc.reshape(KC, P, NSLICES, NS).transpose(2, 1, 0, 3)
        ).reshape(NSLICES, P, KC * NS)
        in_maps.append({"xs": xs, "wct": wct})

    res = run_bass_kernel_spmd(nc, in_maps, core_ids=list(range(8)))

    # ---- gather + fold on host
    y2 = np.empty((n, COUT, 16384), dtype=np.float32)
    for core in range(8):
        b, half = core // 2, core % 2
        arr = res.results[core]["y2"]  # [NTILES, P, MC, NT]
        y2[b, :, half * LSH:(half + 1) * LSH] = (
            arr.transpose(2, 1, 0, 3).reshape(COUT, LSH).astype(np.float32)
        )
        tail = res.results[core]["y2t"]  # [P, MC, NTAIL] bf16
        y2[b, :, (half + 1) * LSH - NTAIL:(half + 1) * LSH] = (
            tail.transpose(1, 0, 2).reshape(COUT, NTAIL).astype(np.float32)
        )

    # bias epilogue (b1/b2 are zeros in this problem; exact otherwise)
    v = W2.astype(np.float64) @ b1.astype(np.float64) + b2.astype(np.float64)
    if np.any(v):
        y2 += v.astype(np.float32)[None, :, None]

    out = y2.reshape(n, c, 2, 2, 128, 128).transpose(0, 1, 4, 2, 5, 3)
    return np.ascontiguousarray(out).reshape(n, c, 256, 256)


# revision 25
# speedup vs baseline: 4.4527x; 1.0032x over previous
"""M2MRF module as a single collapsed GEMM on 8 TRN2 NeuronCores.

The reference is fold(W2 @ (W1 @ unfold(x) + b1) + b2) -- two chained
linear maps with NO nonlinearity between them, so the device only needs
the collapsed weight Wc = W2 @ W1 (precomputed on host in float64):

    cols  = unfold(x[b], k=4, s=4)        # [1024, 16384]
    y2    = Wc @ cols                     # [256, 16384]  (bias via host epilogue)
    out[b] = fold(y2, k=2, s=2)           # [64, 256, 256]

This is a 5x FLOP reduction vs the chained two-GEMM formulation.

Sharding: 8 cores = 4 batches x 2 L-halves (L = 16384 patch positions).
Each core runs one GEMM (256 x 1024 x 8192) in bf16 with fp32 PSUM
accumulation; output returns in bf16 (error budget allows it).

Schedule notes (from timeline analysis):
  - Each DMA holds the global HWDGE descriptor generator ~650ns, so
    inputs are packed into 256-column slices each carrying all 8
    K-chunks (one DMA per slice), 1 weight DMA, ~17 output DMAs.
  - The PE p-state ramp (1.2 GHz for the first 3us after any idle
    period) is bridged with warmup matmuls on a zeroed tile so real
    matmuls all run at 2.4 GHz.
  - 256-column slices let compute start ~5us in and keep slice arrival
    (1.46us) ahead of consumption (1.71us).
  - The last 512-tile is split in two so the final copy+DMA tail is
    short.

Per-core roofline: 2*8*8192 = 131072 PE cycles @ 2.4 GHz = 54.6 us
compute; 21.5 MB total DMA @ 360 GB/s = 59.7 us bus (finishes early,
not critical). Target ~= 5 + 54.6 + 3.8 = 63 us.
"""
import sys

sys.path.insert(0, "/opt/trn_rl_repo")

import numpy as np
import ml_dtypes

import concourse.bass as bass
import concourse.bacc as bacc
import concourse.mybir as mybir
import concourse.tile as tile
from concourse.bass_utils import run_bass_kernel_spmd

P = 128
NT = 512            # PSUM tile free dim
NS = 256            # input slice cols
LSH = 8192          # L per core
NTILES = LSH // NT  # 16
NSLICES = LSH // NS # 32
KC = 8              # 1024 / 128 contraction chunks
MC = 2              # 256 / 128 output chunks
COUT = 256

WARMUP_FULL = 38    # warmup matmuls of 128 cols
NTAIL = 128         # final block: own PSUM tile, short copy+DMA chain

_BF16 = ml_dtypes.bfloat16


def _build_nc(warmup_full=WARMUP_FULL):
    nc = bacc.Bacc("TRN2", target_bir_lowering=False)
    # xs[s][p, k*NS+j] = cols[k*128+p, s*NS+j]: one DMA per column slice s
    # delivers the slice of every contraction chunk k.
    xs_dram = nc.dram_tensor("xs", [NSLICES, P, KC * NS], mybir.dt.bfloat16,
                             kind="ExternalInput")
    # wct[m][p, k, j] = Wc[m*128+j, k*128+p]: contiguous 2KB per partition
    # per m-half, DMA'd separately so compute can start after w_m0 + s0.
    wct_dram = nc.dram_tensor("wct", [MC, P, KC * P], mybir.dt.bfloat16,
                              kind="ExternalInput")
    # y2[t, p, m, j] = y2_full[m*128+p, t*NT+j]: contiguous 2KB per partition.
    y2_dram = nc.dram_tensor("y2", [NTILES, P, MC, NT], mybir.dt.bfloat16,
                             kind="ExternalOutput")
    # final NTAIL cols in their own contiguous tensor (512B-elem descriptors)
    y2t_dram = nc.dram_tensor("y2t", [P, MC, NTAIL], mybir.dt.bfloat16,
                              kind="ExternalOutput")

    with tile.TileContext(nc) as tc:
        with (
            tc.tile_pool(name="resident", bufs=1) as res,
            tc.tile_pool(name="ps", bufs=3, space="PSUM") as ps,
            tc.tile_pool(name="psw", bufs=1, space="PSUM") as psw,
        ):
            wz = res.tile([P, NT], mybir.dt.bfloat16, tag="wz")
            wc_sb = res.tile([P, MC, KC, P], mybir.dt.bfloat16, tag="wc")
            xc_sb = res.tile([P, KC, LSH], mybir.dt.bfloat16, tag="xc")
            # All output tiles resident so compute never stalls on output
            # DMA drain (output transfers queue behind input transfers on
            # the DMA bus).
            o_sb = res.tile([P, NTILES, MC, NT], mybir.dt.bfloat16, tag="o")

            # PE warmup: matmuls on a zeroed tile bridge the p-state ramp
            # so the PE is at 2.4 GHz when the first real operands land.
            # Small memset -> warmup starts ~1.1us; 128-col warmups keep the
            # PE busy until the first input slice lands ~5.8us.
            nc.vector.memset(wz[:, 0:P], 0.0)
            pw = psw.tile([P, NT], mybir.dt.float32, tag="pw")
            for i in range(warmup_full):
                nc.tensor.matmul(pw[:, 0:P], wz[:, 0:P], wz[:, 0:P],
                                 start=True, stop=True)

            # Order: w_m0, s0, w_m1, s1, s2, ... -- the first matmul group
            # (m0 over slice 0) starts as soon as w_m0+s0 land; w_m1 lands
            # on the bus right before the m1 group first needs it.
            nc.sync.dma_start(
                wc_sb[:, 0],
                wct_dram.ap()[0].rearrange("p (k j) -> p k j", k=KC))
            csl = slice(0, NS)
            nc.sync.dma_start(
                xc_sb[:, :, csl],
                xs_dram.ap()[0].rearrange("p (k j) -> p k j", k=KC),
            )
            nc.sync.dma_start(
                wc_sb[:, 1],
                wct_dram.ap()[1].rearrange("p (k j) -> p k j", k=KC))
            for s in range(1, NSLICES):
                csl = slice(s * NS, (s + 1) * NS)
                nc.sync.dma_start(
                    xc_sb[:, :, csl],
                    xs_dram.ap()[s].rearrange("p (k j) -> p k j", k=KC),
                )

            # Compute in 256-col blocks; psum tiles span 512 cols (2 blocks).
            for t in range(NTILES - 1):
                pt = [ps.tile([P, NT], mybir.dt.float32, tag=f"ps{m}",
                              name=f"pt{t}_{m}")
                      for m in range(MC)]
                for h in range(2):  # half = 256-col block = one input slice
                    hsl = slice(t * NT + h * NS, t * NT + (h + 1) * NS)
                    psl = slice(h * NS, (h + 1) * NS)
                    for m in range(MC):
                        for k in range(KC):
                            nc.tensor.matmul(
                                pt[m][:, psl],
                                wc_sb[:, m, k, :],
                                xc_sb[:, k, hsl],
                                start=(k == 0),
                                stop=(k == KC - 1),
                            )
                for m in range(MC):
                    nc.any.tensor_copy(out=o_sb[:, t, m], in_=pt[m][:])
                nc.sync.dma_start(y2_dram.ap()[t], o_sb[:, t])

            # Last tile: a 384-col block through the normal copy path, then
            # a final NTAIL-col block accumulated in its own PSUM tile and
            # DMA'd straight to DRAM (fp32) so the tail is one short chain.
            t = NTILES - 1
            NH = NT - NTAIL  # 384
            pt = [ps.tile([P, NT], mybir.dt.float32, tag=f"ps{m}",
                          name=f"ptl{m}")
                  for m in range(MC)]
            for m in range(MC):
                for k in range(KC):
                    nc.tensor.matmul(
                        pt[m][:, 0:NH],
                        wc_sb[:, m, k, :],
                        xc_sb[:, k, t * NT:t * NT + NH],
                        start=(k == 0),
                        stop=(k == KC - 1),
                    )
            nc.scalar.copy(out=o_sb[:, t, 0, 0:NH], in_=pt[0][:, 0:NH])
            nc.vector.tensor_copy(out=o_sb[:, t, 1, 0:NH], in_=pt[1][:, 0:NH])
            nc.sync.dma_start(y2_dram.ap()[t, :, :, 0:NH], o_sb[:, t, :, 0:NH])

            ptail = psw.tile([P, MC, NTAIL], mybir.dt.float32, tag="ptail")
            ot_sb = res.tile([P, MC, NTAIL], mybir.dt.bfloat16, tag="ot")
            for m in range(MC):
                for k in range(KC):
                    nc.tensor.matmul(
                        ptail[:, m, :],
                        wc_sb[:, m, k, :],
                        xc_sb[:, k, LSH - NTAIL:LSH],
                        start=(k == 0),
                        stop=(k == KC - 1),
                    )
            nc.vector.tensor_copy(out=ot_sb[:], in_=ptail[:])
            nc.sync.dma_start(y2t_dram.ap(), ot_sb[:])

    nc.finalize()
    return nc


_NC_CACHE = None


def kernel(x, W1, b1, W2, b2):
    global _NC_CACHE
    x = np.asarray(x)
    W1, b1 = np.asarray(W1), np.asarray(b1)
    W2, b2 = np.asarray(W2), np.asarray(b2)
    n, c, h, w = x.shape  # 4, 64, 512, 512

    # ---- host unfold: cols[b, c*16+kh*4+kw, ph*128+pw] = x[b,c,ph*4+kh,pw*4+kw]
    xb = x.astype(_BF16)
    cols = xb.reshape(n, c, 128, 4, 128, 4).transpose(0, 1, 3, 5, 2, 4)
    cols = np.ascontiguousarray(cols).reshape(n, 1024, 16384)

    # ---- collapsed weight (exact in f64, one bf16 rounding)
    Wc = W2.astype(np.float64) @ W1.astype(np.float64)  # [256, 1024]
    wct = np.ascontiguousarray(
        Wc.reshape(MC, P, KC, P).transpose(0, 3, 2, 1)
    ).reshape(MC, P, KC * P).astype(_BF16)  # [m, p, k, j]

    if _NC_CACHE is None:
        _NC_CACHE = _build_nc()
    nc = _NC_CACHE

    in_maps = []
    for core in range(8):
        b, half = core // 2, core % 2
        xc = cols[b, :, half * LSH:(half + 1) * LSH]  # [1024, LSH]
        # [NSLICES, P, KC, NS]: xs[s, p, k, j] = xc[k*128+p, s*NS+j]
        xs = np.ascontiguousarray(
            xc.reshape(KC, P, NSLICES, NS).transpose(2, 1, 0, 3)
        ).reshape(NSLICES, P, KC * NS)
        in_maps.append({"xs": xs, "wct": wct})

    res = run_bass_kernel_spmd(nc, in_maps, core_ids=list(range(8)))

    # ---- gather + fold on host
    y2 = np.empty((n, COUT, 16384), dtype=np.float32)
    for core in range(8):
        b, half = core // 2, core % 2
        arr = res.results[core]["y2"]  # [NTILES, P, MC, NT]
        y2[b, :, half * LSH:(half + 1) * LSH] = (
            arr.transpose(2, 1, 0, 3).reshape(COUT, LSH).astype(np.float32)
        )
        tail = res.results[core]["y2t"]  # [P, MC, NTAIL] bf16
        y2[b, :, (half + 1) * LSH - NTAIL:(half + 1) * LSH] = (
            tail.transpose(1, 0, 2).reshape(COUT, NTAIL).astype(np.float32)
        )

    # bias epilogue (b1/b2 are zeros in this problem; exact otherwise)
    v = W2.astype(np.float64) @ b1.astype(np.float64) + b2.astype(np.float64)
    if np.any(v):
        y2 += v.astype(np.float32)[None, :, None]

    out = y2.reshape(n, c, 2, 2, 128, 128).transpose(0, 1, 4, 2, 5, 3)
    return np.ascontiguousarray(out).reshape(n, c, 256, 256)
